# revision 18
# baseline (speedup 1.0000x reference)
"""Trainium2 Bass kernel for the CNF (RK4 + Hutchinson divergence) problem.

Strategy: pure data parallel over batch (4096 -> 512 per core on 8 cores).
Everything resident in SBUF; activations stored feature-major [feat, batch]
so MLP weights are used directly as the stationary matmul operand and biases
become per-partition scalars fused into the scalar-engine activation op.
Matmuls run in float32r (full PE rate, ~11-bit mantissa); fp32 elsewhere.

Tangent (Hutchinson JVP) pass carries a sign-flip: dh'_l = (h_l^2 - 1) * d'_l
computed in ONE fused DVE op (scalar_tensor_tensor); the flipped signs cancel
pairwise and the residual sign is folded into the divergence-reduction
coefficient vector (cvec = (h/6)*c, applied as the colsum-matmul lhsT).

Pipeline notes (from perfetto traces):
- matmul issue rate ~234 ns at N=512 is the roofline; the kernel is PE-bound
  with ACT (tanh/square) as the secondary ceiling.
- layer-0 tangent matmuls depend only on eps, so they are hoisted into the
  previous stage's tail to cover the z-update dependency stall.
- the divergence colsum matmul is deferred past the NEXT stage's layer-0
  forward matmuls (its result only feeds lp, which is off the critical path).
- squares run alternately on ScalarE and GpSimdE (otherwise the ACT FIFO
  backlog delays the tangent gates and with them the L3 tangent matmuls).
- all DMA is 5 instructions total (packed weights, eps head, eps tail, two
  outputs): every DMA instruction occupies a distinct semaphore lane and the
  kernel-tail drain waits on every lane ever used.
"""

import numpy as np

import concourse.bass as bass
import concourse.bacc as bacc
import concourse.mybir as mybir
import concourse.tile as tile
from concourse.bass_utils import run_bass_kernel_spmd
from contextlib import ExitStack

# Problem constants (hardcoded per the grading contract).
B, DIM, COND, HID, STEPS, TSAMP = 4096, 64, 64, 512, 10, 1
NCORES = 8
BC = B // NCORES            # batch per core
NINT = STEPS - 1            # RK4 intervals
NSTG = 4 * NINT             # total MLP evals
NJ = HID // 128             # feature tiles per hidden layer
H = np.float64(-1.0) / NINT  # integrator step

F32 = mybir.dt.float32
F32R = mybir.dt.float32r
AF = mybir.ActivationFunctionType
OP = mybir.AluOpType

# Packed-weights column layout: [w0 512 | w1 2048 | w2 2048 | w3 256 |
#   tb 144 | bb 8 | b3 1 | cvec 2 | zc0 512]
_C_W0 = 0
_C_W1 = _C_W0 + 512
_C_W2 = _C_W1 + 2048
_C_W3 = _C_W2 + 2048
_C_TB = _C_W3 + 256
_C_BB = _C_TB + NSTG * 4
_C_B3 = _C_BB + 8
_C_B3H = _C_B3 + 1   # (h/2)*b3s
_C_B3F = _C_B3H + 1  # h*b3s
_C_CV = _C_B3F + 1
_C_ZC = _C_CV + 2
WPACK_COLS = _C_ZC + 512


def _stage_times():
    h = H
    tspan = np.linspace(1.0, 0.0, NINT + 1)[:-1]
    ts = []
    for i in range(NINT):
        t = tspan[i]
        ts += [t, t + 0.5 * h, t + 0.5 * h, t + h]
    return ts  # 36 stage times, float64


def build_nc():
    nc = bacc.Bacc("TRN2", target_bir_lowering=False, debug=False,
                   num_devices=NCORES)

    wpack_d = nc.dram_tensor("wpack", [128, WPACK_COLS], F32,
                             kind="ExternalInput").ap()
    eps_d = nc.dram_tensor("epsT", [64, NSTG * BC], F32,
                           kind="ExternalInput").ap()
    zout_d = nc.dram_tensor("z_out", [64, BC], F32, kind="ExternalOutput").ap()
    lpout_d = nc.dram_tensor("lp_out", [1, BC], F32, kind="ExternalOutput").ap()

    with tile.TileContext(nc) as tc, ExitStack() as ctx:
        wpool = ctx.enter_context(tc.tile_pool(name="wpool", bufs=1))
        hpool = ctx.enter_context(tc.tile_pool(name="hpool", bufs=1))
        psum = ctx.enter_context(tc.tile_pool(name="psum", bufs=1, space="PSUM"))

        # ---- one-time loads & conversions ----
        wf = wpool.tile([128, WPACK_COLS], F32, tag="wf")
        nc.gpsimd.dma_start(wf[:], wpack_d[:])
        epsf = wpool.tile([64, NSTG * BC], F32, tag="epsf")
        nc.gpsimd.dma_start(epsf[:, 0:BC], eps_d[:, 0:BC])
        nc.gpsimd.dma_start(epsf[:, BC:], eps_d[:, BC:])

        # float32r copies of everything that feeds a matmul (the fp32r
        # verifier is location-based: the f32r buffer's only writer must be
        # a rounding engine op, so DMA-fed data needs a separate copy)
        wr = wpool.tile([128, _C_TB], F32R, tag="wr")  # w0|w1|w2|w3
        nc.vector.tensor_copy(wr[:, 0:_C_W1], wf[:, 0:_C_W1])
        nc.vector.tensor_copy(wr[:, _C_W1:_C_TB], wf[:, _C_W1:_C_TB])
        cvec_r = wpool.tile([64, 2], F32R, tag="cvr")
        nc.vector.tensor_copy(cvec_r[:], wf[0:64, _C_CV:_C_CV + 2])

        def w0j(j):
            return wr[:, _C_W0 + j * 128: _C_W0 + (j + 1) * 128]

        def w0j64(j):
            return wr[0:64, _C_W0 + j * 128: _C_W0 + (j + 1) * 128]

        def w12(base, k, j):
            c = base + k * 512 + j * 128
            return wr[:, c:c + 128]

        def w3k(k):
            return wr[:, _C_W3 + k * 64: _C_W3 + (k + 1) * 64]

        def tb_ap(s, j):
            c = _C_TB + 4 * s + j
            return wf[:, c:c + 1]

        def bb_ap(i):
            return wf[:, _C_BB + i:_C_BB + i + 1]

        b3_ap = wf[0:64, _C_B3:_C_B3 + 1]
        b3h_ap = wf[0:64, _C_B3H:_C_B3H + 1]
        b3f_ap = wf[0:64, _C_B3F:_C_B3F + 1]

        # persistent states
        zcx = wpool.tile([128, BC], F32R, tag="zcx")   # rows 0:64 z, 64:128 ctx
        nc.vector.tensor_copy(zcx[:], wf[:, _C_ZC:_C_ZC + 512])

        zpp = [wpool.tile([64, BC], F32, tag=f"z{i}", name=f"z{i}")
               for i in range(2)]
        nc.vector.tensor_copy(zpp[0][:], wf[0:64, _C_ZC:_C_ZC + 512])
        lpp = [wpool.tile([1, BC], F32, tag=f"lp{i}", name=f"lp{i}")
               for i in range(2)]
        nc.vector.memset(lpp[0][:], 0.0)

        state = {
            "z_cur": zpp[0], "z_nxt": zpp[1],
            "lp_cur": lpp[0], "lp_nxt": lpp[1],
        }
        coeffs = [1.0, 2.0, 2.0, 1.0]
        er_tiles = {}

        def get_er(s):
            # f32r copy of the stage-s eps slice (prefetched a stage ahead)
            if s not in er_tiles and s < NSTG:
                er = hpool.tile([64, BC], F32R, tag="er", bufs=3, name="er")
                nc.vector.tensor_copy(er[:], epsf[:, s * BC:(s + 1) * BC])
                er_tiles[s] = er
            return er_tiles.get(s)

        def emit_psds(s):
            # layer-0 tangent matmuls: independent of z, they keep the PE
            # busy across stage-boundary dependency stalls
            out = []
            for j in range(NJ):
                psd = psum.tile([128, BC], F32, tag="ps", bufs=8, name="ps")
                nc.tensor.matmul(psd[:], w0j64(j), get_er(s)[:],
                                 start=True, stop=True)
                out.append(psd)
            return out

        def emit_square(hj, j):
            qj = hpool.tile([128, BC], F32, tag="q", bufs=4, name="q")
            if j % 2 == 0:
                nc.scalar.square(qj[:], hj[:])
            else:
                nc.gpsimd.tensor_tensor(qj[:], hj[:], hj[:], OP.mult)
            return qj

        def emit_div(pend):
            # divergence colsum with the RK coeff folded into the lhsT
            prod, st = pend
            psv = psum.tile([1, BC], F32, tag="ps", bufs=8, name="ps")
            cv = cvec_r[:, 0:1] if coeffs[st] == 1.0 else cvec_r[:, 1:2]
            nc.tensor.matmul(psv[:], cv, prod[:], start=True, stop=True)
            nc.vector.tensor_add(state["lp_nxt"][:], state["lp_cur"][:], psv[:])
            state["lp_cur"], state["lp_nxt"] = state["lp_nxt"], state["lp_cur"]

        psds_cur = emit_psds(0)
        pending_div = None
        t2 = None
        f_tiles = []

        for s in range(NSTG):
            it, st = divmod(s, 4)
            e_f = epsf[:, s * BC:(s + 1) * BC]
            psds = psds_cur

            # ---- layer 0 forward ----
            h_prev = []
            for j in range(NJ):
                ps = psum.tile([128, BC], F32, tag="ps", bufs=8, name="ps")
                nc.tensor.matmul(ps[:], w0j(j), zcx[:], start=True, stop=True)
                hj = hpool.tile([128, BC], F32R, tag="h", bufs=8, name="h")
                nc.scalar.activation(hj[:], ps[:], AF.Tanh, bias=tb_ap(s, j))
                h_prev.append(hj)
            # previous stage's divergence matmul goes here: its result only
            # feeds lp, so it must not gate this stage's layer-0 matmuls
            if pending_div is not None:
                emit_div(pending_div)
                pending_div = None
            # pre-biased z tiles for the direct-from-PSUM stage updates
            if st == 0:
                zbh = hpool.tile([64, BC], F32, tag="zbh", bufs=2, name="zbh")
                nc.vector.tensor_scalar(zbh[:], state["z_cur"][:], b3h_ap,
                                        None, OP.add)
                zbf = hpool.tile([64, BC], F32, tag="zbf", bufs=2, name="zbf")
                nc.vector.tensor_scalar(zbf[:], state["z_cur"][:], b3f_ap,
                                        None, OP.add)
            d_prev = []
            for j in range(NJ):
                qj = emit_square(h_prev[j], j)
                dj = hpool.tile([128, BC], F32R, tag="d", bufs=8, name="d")
                # dh' = (h^2 - 1) * d   (sign flip, cancels next layer)
                nc.vector.scalar_tensor_tensor(
                    dj[:], qj[:], -1.0, psds[j][:], OP.add, OP.mult)
                d_prev.append(dj)

            # ---- layers 1, 2 ----
            # k-outer / j-inner: the PE consumes each h[k] / dh[k] tile as
            # ACT/GPS/DVE produce it, instead of stalling on the last one
            l2_q = l2_pt = None
            for li, base in enumerate((_C_W1, _C_W2)):
                ps_f = [psum.tile([128, BC], F32, tag="ps", bufs=8,
                                  name="ps") for _ in range(NJ)]
                for k in range(NJ):
                    for j in range(NJ):
                        nc.tensor.matmul(ps_f[j][:], w12(base, k, j),
                                         h_prev[k][:],
                                         start=(k == 0), stop=(k == NJ - 1))
                h_new, q_new = [], []
                for j in range(NJ):
                    hj = hpool.tile([128, BC], F32R, tag="h", bufs=8, name="h")
                    nc.scalar.activation(hj[:], ps_f[j][:], AF.Tanh,
                                         bias=bb_ap(4 * li + j))
                    h_new.append(hj)
                    q_new.append(emit_square(hj, j))
                ps_t = [psum.tile([128, BC], F32, tag="ps", bufs=8,
                                  name="ps") for _ in range(NJ)]
                for k in range(NJ):
                    for j in range(NJ):
                        nc.tensor.matmul(ps_t[j][:], w12(base, k, j),
                                         d_prev[k][:],
                                         start=(k == 0), stop=(k == NJ - 1))
                if li == 0:
                    d_new = []
                    for j in range(NJ):
                        dj = hpool.tile([128, BC], F32R, tag="d", bufs=8,
                                        name="d")
                        nc.vector.scalar_tensor_tensor(
                            dj[:], q_new[j][:], -1.0, ps_t[j][:],
                            OP.add, OP.mult)
                        d_new.append(dj)
                    h_prev, d_prev = h_new, d_new
                else:
                    # layer 2: delay the tangent-gate DVE ops until after
                    # f / z-update are queued (they gate the next stage)
                    l2_q, l2_pt = q_new, ps_t
                    h_prev = h_new

            # ---- layer 3 forward + z chain (critical path) ----
            psf = psum.tile([64, BC], F32, tag="ps", bufs=8, name="ps")
            for k in range(NJ):
                nc.tensor.matmul(psf[:], w3k(k), h_prev[k][:],
                                 start=(k == 0), stop=(k == NJ - 1))
            # stage update straight from PSUM: z_s+1 = psf*c + (z + c*b3s);
            # f = psf + b3 materializes OFF the critical path (only needed
            # for the RK combine, and not at all for stage 4)
            z_cur = state["z_cur"]
            if st < 3:
                if st < 2:
                    nc.vector.scalar_tensor_tensor(
                        zcx[0:64, :], psf[:], float(0.5 * H), zbh[:],
                        OP.mult, OP.add)
                else:
                    nc.vector.scalar_tensor_tensor(
                        zcx[0:64, :], psf[:], float(H), zbf[:],
                        OP.mult, OP.add)
                f = hpool.tile([64, BC], F32, tag="f", bufs=3, name="f")
                nc.vector.tensor_scalar(f[:], psf[:], b3_ap, None, OP.add)
                f_tiles.append(f)
            else:
                t3 = hpool.tile([64, BC], F32, tag="t3", bufs=2, name="t3")
                nc.vector.tensor_add(t3[:], t2b[:], psf[:])
                nc.vector.scalar_tensor_tensor(
                    zcx[0:64, :], t3[:], float(H / 6.0), z_cur[:],
                    OP.mult, OP.add)
                nc.vector.scalar_tensor_tensor(
                    state["z_nxt"][:], t3[:], float(H / 6.0), z_cur[:],
                    OP.mult, OP.add)
                state["z_cur"], state["z_nxt"] = state["z_nxt"], state["z_cur"]

            # layer-2 tangent gates (feed the L3 tangent matmuls)
            d_prev = []
            for j in range(NJ):
                dj = hpool.tile([128, BC], F32R, tag="d", bufs=8, name="d")
                nc.vector.scalar_tensor_tensor(
                    dj[:], l2_q[j][:], -1.0, l2_pt[j][:], OP.add, OP.mult)
                d_prev.append(dj)

            # hoisted next-stage tangent matmuls cover the dh2/z waits
            if s + 1 < NSTG:
                psds_cur = emit_psds(s + 1)

            # ---- layer 3 tangent + divergence product ----
            psj = psum.tile([64, BC], F32, tag="ps", bufs=8, name="ps")
            for k in range(NJ):
                nc.tensor.matmul(psj[:], w3k(k), d_prev[k][:],
                                 start=(k == 0), stop=(k == NJ - 1))
            prod = hpool.tile([64, BC], F32R, tag="prod", bufs=3, name="prod")
            nc.vector.tensor_tensor(prod[:], psj[:], e_f, OP.mult)
            pending_div = (prod, st)

            # start the RK4 combine early to shorten the interval tail
            if st == 2:
                t1 = hpool.tile([64, BC], F32, tag="t1", bufs=2, name="t1")
                nc.vector.tensor_add(t1[:], f_tiles[1][:], f_tiles[2][:])
                t2 = hpool.tile([64, BC], F32, tag="t2", bufs=2, name="t2")
                nc.vector.scalar_tensor_tensor(
                    t2[:], t1[:], 2.0, f_tiles[0][:], OP.mult, OP.add)
                t2b = hpool.tile([64, BC], F32, tag="t2b", bufs=2, name="t2b")
                nc.vector.tensor_scalar(t2b[:], t2[:], b3_ap, None, OP.add)
            if st == 3:
                f_tiles = []

        emit_div(pending_div)

        nc.gpsimd.dma_start(zout_d[:], state["z_cur"][:])
        nc.gpsimd.dma_start(lpout_d[:], state["lp_cur"][:])

    nc.compile()
    return nc


def _prep_host(inputs):
    """Host-side packing: weights/biases shared across cores, per-core slices."""
    f32 = np.float32
    x = np.asarray(inputs["x"], f32)
    context = np.asarray(inputs["context"], f32)
    W0 = np.asarray(inputs["W0"], f32)
    b0 = np.asarray(inputs["b0"], f32)
    W1 = np.asarray(inputs["W1"], f32)
    b1 = np.asarray(inputs["b1"], f32)
    W2 = np.asarray(inputs["W2"], f32)
    b2 = np.asarray(inputs["b2"], f32)
    W3 = np.asarray(inputs["W3"], f32)
    b3 = np.asarray(inputs["b3"], f32)
    out_scale = np.asarray(inputs["out_scale"], f32)
    eps = np.asarray(inputs["eps"], f32)

    W3s = (W3 * out_scale).astype(f32)
    b3s = (b3 * out_scale).astype(f32)

    wpack = np.zeros((128, WPACK_COLS), f32)
    wpack[:, _C_W0:_C_W0 + 512] = W0[:128]
    wpack[:, _C_W1:_C_W1 + 2048] = W1.reshape(4, 128, 512).transpose(1, 0, 2).reshape(128, 2048)
    wpack[:, _C_W2:_C_W2 + 2048] = W2.reshape(4, 128, 512).transpose(1, 0, 2).reshape(128, 2048)
    wpack[:, _C_W3:_C_W3 + 256] = W3s.reshape(4, 128, 64).transpose(1, 0, 2).reshape(128, 256)
    w0t = W0[DIM + COND].astype(np.float64)
    for s, t in enumerate(_stage_times()):
        wpack[:, _C_TB + 4 * s:_C_TB + 4 * s + 4] = \
            (b0.astype(np.float64) + t * w0t).astype(f32).reshape(4, 128).T
    wpack[:, _C_BB:_C_BB + 4] = b1.reshape(4, 128).T
    wpack[:, _C_BB + 4:_C_BB + 8] = b2.reshape(4, 128).T
    wpack[0:64, _C_B3] = b3s
    wpack[0:64, _C_B3H] = (0.5 * H * b3s.astype(np.float64)).astype(f32)
    wpack[0:64, _C_B3F] = (H * b3s.astype(np.float64)).astype(f32)
    # divergence coeffs: lp += cvec^T (jv' * e) with jv' = -jv:
    # lp_contrib = (h/6)*c*k_l = -(h/6)*c*div = +(h/6)*c*div'  => (H/6)*c.
    wpack[0:64, _C_CV] = f32(H / 6.0)
    wpack[0:64, _C_CV + 1] = f32((H / 6.0) * 2.0)

    in_maps = []
    for c in range(NCORES):
        sl = slice(c * BC, (c + 1) * BC)
        wp = wpack.copy()
        wp[0:64, _C_ZC:_C_ZC + 512] = x[sl].T
        wp[64:128, _C_ZC:_C_ZC + 512] = context[sl].T
        epsT = np.ascontiguousarray(
            eps[:, :, 0, sl, :].reshape(NSTG, BC, 64).transpose(2, 0, 1).reshape(64, NSTG * BC))
        in_maps.append({"wpack": wp, "epsT": epsT})
    return in_maps


_NC_CACHE = None


def kernel(**inputs):
    global _NC_CACHE
    if _NC_CACHE is None:
        _NC_CACHE = build_nc()
    nc = _NC_CACHE
    in_maps = _prep_host(inputs)
    res = run_bass_kernel_spmd(nc, in_maps, core_ids=list(range(NCORES)))
    z1 = np.empty((B, DIM), np.float32)
    lp1 = np.empty((B, 1), np.float32)
    for c in range(NCORES):
        sl = slice(c * BC, (c + 1) * BC)
        z1[sl] = res.results[c]["z_out"].T
        lp1[sl] = res.results[c]["lp_out"].T
    return z1, lp1


# revision 19
# speedup vs baseline: 1.1946x; 1.1946x over previous
"""Trainium2 Bass kernel for the CNF (RK4 + Hutchinson divergence) problem.

Strategy: pure data parallel over batch (4096 -> 512 per core on 8 cores).
Everything resident in SBUF; activations stored feature-major [feat, batch]
so MLP weights are used directly as the stationary matmul operand and biases
become per-partition scalars fused into the scalar-engine activation op.
Matmuls run in float32r (full PE rate, ~11-bit mantissa); fp32 elsewhere.

Tangent (Hutchinson JVP) pass carries a sign-flip: dh'_l = (h_l^2 - 1) * d'_l
computed in ONE fused DVE op (scalar_tensor_tensor); the flipped signs cancel
pairwise and the residual sign is folded into the divergence-reduction
coefficient vector (cvec = (h/6)*c, applied as the colsum-matmul lhsT).

Pipeline notes (from perfetto traces):
- matmul issue rate ~234 ns at N=512 is the roofline; the kernel is PE-bound
  with ACT (tanh/square) as the secondary ceiling.
- layer-0 tangent matmuls depend only on eps, so they are hoisted into the
  previous stage's tail to cover the z-update dependency stall.
- the divergence colsum matmul is deferred past the NEXT stage's layer-0
  forward matmuls (its result only feeds lp, which is off the critical path).
- squares run alternately on ScalarE and GpSimdE (otherwise the ACT FIFO
  backlog delays the tangent gates and with them the L3 tangent matmuls).
- all DMA is 5 instructions total (packed weights, eps head, eps tail, two
  outputs): every DMA instruction occupies a distinct semaphore lane and the
  kernel-tail drain waits on every lane ever used.
"""

import numpy as np

import concourse.bass as bass
import concourse.bacc as bacc
import concourse.mybir as mybir
import concourse.tile as tile
from concourse.bass_utils import run_bass_kernel_spmd
from contextlib import ExitStack

# Problem constants (hardcoded per the grading contract).
B, DIM, COND, HID, STEPS, TSAMP = 4096, 64, 64, 512, 10, 1
NCORES = 8
BC = B // NCORES            # batch per core
NINT = STEPS - 1            # RK4 intervals
NSTG = 4 * NINT             # total MLP evals
NJ = HID // 128             # feature tiles per hidden layer
H = np.float64(-1.0) / NINT  # integrator step

F32 = mybir.dt.float32
F32R = mybir.dt.float32r
AF = mybir.ActivationFunctionType
OP = mybir.AluOpType

# Packed-weights column layout: [w0 512 | w1 2048 | w2 2048 | w3 256 |
#   tb 144 | bb 8 | b3 1 | cvec 2 | zc0 512]
_C_W0 = 0
_C_W1 = _C_W0 + 512
_C_W2 = _C_W1 + 2048
_C_W3 = _C_W2 + 2048
_C_TB = _C_W3 + 256
_C_BB = _C_TB + NSTG * 4
_C_B3 = _C_BB + 8
_C_CV = _C_B3 + 1
_C_ZC = _C_CV + 2
WPACK_COLS = _C_ZC + 512


def _stage_times():
    h = H
    tspan = np.linspace(1.0, 0.0, NINT + 1)[:-1]
    ts = []
    for i in range(NINT):
        t = tspan[i]
        ts += [t, t + 0.5 * h, t + 0.5 * h, t + h]
    return ts  # 36 stage times, float64


def build_nc():
    nc = bacc.Bacc("TRN2", target_bir_lowering=False, debug=False,
                   num_devices=NCORES)

    wpack_d = nc.dram_tensor("wpack", [128, WPACK_COLS], F32,
                             kind="ExternalInput").ap()
    eps_d = nc.dram_tensor("epsT", [64, NSTG * BC], F32,
                           kind="ExternalInput").ap()
    zout_d = nc.dram_tensor("z_out", [64, BC], F32, kind="ExternalOutput").ap()
    lpout_d = nc.dram_tensor("lp_out", [1, BC], F32, kind="ExternalOutput").ap()

    with tile.TileContext(nc) as tc, ExitStack() as ctx:
        wpool = ctx.enter_context(tc.tile_pool(name="wpool", bufs=1))
        hpool = ctx.enter_context(tc.tile_pool(name="hpool", bufs=1))
        psum = ctx.enter_context(tc.tile_pool(name="psum", bufs=1, space="PSUM"))

        # ---- one-time loads & conversions ----
        wf = wpool.tile([128, WPACK_COLS], F32, tag="wf")
        nc.gpsimd.dma_start(wf[:], wpack_d[:])
        epsf = wpool.tile([64, NSTG * BC], F32, tag="epsf")
        nc.gpsimd.dma_start(epsf[:, 0:BC], eps_d[:, 0:BC])
        nc.gpsimd.dma_start(epsf[:, BC:], eps_d[:, BC:])

        # float32r copies of everything that feeds a matmul (the fp32r
        # verifier is location-based: the f32r buffer's only writer must be
        # a rounding engine op, so DMA-fed data needs a separate copy)
        wr = wpool.tile([128, _C_TB], F32R, tag="wr")  # w0|w1|w2|w3
        nc.vector.tensor_copy(wr[:, 0:_C_W1], wf[:, 0:_C_W1])
        nc.vector.tensor_copy(wr[:, _C_W1:_C_TB], wf[:, _C_W1:_C_TB])
        cvec_r = wpool.tile([64, 2], F32R, tag="cvr")
        nc.vector.tensor_copy(cvec_r[:], wf[0:64, _C_CV:_C_CV + 2])

        def w0j(j):
            return wr[:, _C_W0 + j * 128: _C_W0 + (j + 1) * 128]

        def w0j64(j):
            return wr[0:64, _C_W0 + j * 128: _C_W0 + (j + 1) * 128]

        def w12(base, k, j):
            c = base + k * 512 + j * 128
            return wr[:, c:c + 128]

        def w3k(k):
            return wr[:, _C_W3 + k * 64: _C_W3 + (k + 1) * 64]

        def tb_ap(s, j):
            c = _C_TB + 4 * s + j
            return wf[:, c:c + 1]

        def bb_ap(i):
            return wf[:, _C_BB + i:_C_BB + i + 1]

        b3_ap = wf[0:64, _C_B3:_C_B3 + 1]

        # persistent states
        zcx = wpool.tile([128, BC], F32R, tag="zcx")   # rows 0:64 z, 64:128 ctx
        nc.vector.tensor_copy(zcx[:], wf[:, _C_ZC:_C_ZC + 512])

        zpp = [wpool.tile([64, BC], F32, tag=f"z{i}", name=f"z{i}")
               for i in range(2)]
        nc.vector.tensor_copy(zpp[0][:], wf[0:64, _C_ZC:_C_ZC + 512])
        lpp = [wpool.tile([1, BC], F32, tag=f"lp{i}", name=f"lp{i}")
               for i in range(2)]
        nc.vector.memset(lpp[0][:], 0.0)

        state = {
            "z_cur": zpp[0], "z_nxt": zpp[1],
            "lp_cur": lpp[0], "lp_nxt": lpp[1],
        }
        coeffs = [1.0, 2.0, 2.0, 1.0]
        er_tiles = {}

        def get_er(s):
            # f32r copy of the stage-s eps slice (prefetched a stage ahead)
            if s not in er_tiles and s < NSTG:
                er = hpool.tile([64, BC], F32R, tag="er", bufs=3, name="er")
                nc.vector.tensor_copy(er[:], epsf[:, s * BC:(s + 1) * BC])
                er_tiles[s] = er
            return er_tiles.get(s)

        def emit_psds(s):
            # layer-0 tangent matmuls: independent of z, they keep the PE
            # busy across stage-boundary dependency stalls
            out = []
            for j in range(NJ):
                psd = psum.tile([128, BC], F32, tag="ps", bufs=8, name="ps")
                nc.tensor.matmul(psd[:], w0j64(j), get_er(s)[:],
                                 start=True, stop=True)
                out.append(psd)
            return out

        def emit_square(hj, j):
            qj = hpool.tile([128, BC], F32, tag="q", bufs=4, name="q")
            if j % 2 == 0:
                nc.scalar.square(qj[:], hj[:])
            else:
                nc.gpsimd.tensor_tensor(qj[:], hj[:], hj[:], OP.mult)
            return qj

        def emit_div(pend):
            # divergence colsum with the RK coeff folded into the lhsT
            prod, st = pend
            psv = psum.tile([1, BC], F32, tag="ps", bufs=8, name="ps")
            cv = cvec_r[:, 0:1] if coeffs[st] == 1.0 else cvec_r[:, 1:2]
            nc.tensor.matmul(psv[:], cv, prod[:], start=True, stop=True)
            nc.vector.tensor_add(state["lp_nxt"][:], state["lp_cur"][:], psv[:])
            state["lp_cur"], state["lp_nxt"] = state["lp_nxt"], state["lp_cur"]

        psds_cur = emit_psds(0)
        pending_div = None
        t2 = None
        f_tiles = []

        for s in range(NSTG):
            it, st = divmod(s, 4)
            e_f = epsf[:, s * BC:(s + 1) * BC]
            psds = psds_cur

            # ---- layer 0 forward ----
            h_prev = []
            for j in range(NJ):
                ps = psum.tile([128, BC], F32, tag="ps", bufs=8, name="ps")
                nc.tensor.matmul(ps[:], w0j(j), zcx[:], start=True, stop=True)
                hj = hpool.tile([128, BC], F32R, tag="h", bufs=8, name="h")
                nc.scalar.activation(hj[:], ps[:], AF.Tanh, bias=tb_ap(s, j))
                h_prev.append(hj)
            # previous stage's divergence matmul goes here: its result only
            # feeds lp, so it must not gate this stage's layer-0 matmuls
            if pending_div is not None:
                emit_div(pending_div)
                pending_div = None
            d_prev = []
            for j in range(NJ):
                qj = emit_square(h_prev[j], j)
                dj = hpool.tile([128, BC], F32R, tag="d", bufs=8, name="d")
                # dh' = (h^2 - 1) * d   (sign flip, cancels next layer)
                nc.vector.scalar_tensor_tensor(
                    dj[:], qj[:], -1.0, psds[j][:], OP.add, OP.mult)
                d_prev.append(dj)

            # ---- layers 1, 2 ----
            # k-outer / j-inner: the PE consumes each h[k] / dh[k] tile as
            # ACT/GPS/DVE produce it, instead of stalling on the last one
            l2_q = l2_pt = None
            for li, base in enumerate((_C_W1, _C_W2)):
                ps_f = [psum.tile([128, BC], F32, tag="ps", bufs=8,
                                  name="ps") for _ in range(NJ)]
                for k in range(NJ):
                    for j in range(NJ):
                        nc.tensor.matmul(ps_f[j][:], w12(base, k, j),
                                         h_prev[k][:],
                                         start=(k == 0), stop=(k == NJ - 1))
                h_new, q_new = [], []
                for j in range(NJ):
                    hj = hpool.tile([128, BC], F32R, tag="h", bufs=8, name="h")
                    nc.scalar.activation(hj[:], ps_f[j][:], AF.Tanh,
                                         bias=bb_ap(4 * li + j))
                    h_new.append(hj)
                    q_new.append(emit_square(hj, j))
                ps_t = [psum.tile([128, BC], F32, tag="ps", bufs=8,
                                  name="ps") for _ in range(NJ)]
                for k in range(NJ):
                    for j in range(NJ):
                        nc.tensor.matmul(ps_t[j][:], w12(base, k, j),
                                         d_prev[k][:],
                                         start=(k == 0), stop=(k == NJ - 1))
                if li == 0:
                    d_new = []
                    for j in range(NJ):
                        dj = hpool.tile([128, BC], F32R, tag="d", bufs=8,
                                        name="d")
                        nc.vector.scalar_tensor_tensor(
                            dj[:], q_new[j][:], -1.0, ps_t[j][:],
                            OP.add, OP.mult)
                        d_new.append(dj)
                    h_prev, d_prev = h_new, d_new
                else:
                    # layer 2: delay the tangent-gate DVE ops until after
                    # f / z-update are queued (they gate the next stage)
                    l2_q, l2_pt = q_new, ps_t
                    h_prev = h_new

            # ---- layer 3 forward + z chain (critical path) ----
            psf = psum.tile([64, BC], F32, tag="ps", bufs=8, name="ps")
            for k in range(NJ):
                nc.tensor.matmul(psf[:], w3k(k), h_prev[k][:],
                                 start=(k == 0), stop=(k == NJ - 1))
            # f on DVE: the ACT FIFO is backlogged with layer-2 tanh here
            f = hpool.tile([64, BC], F32, tag="f", bufs=5, name="f")
            nc.vector.tensor_scalar(f[:], psf[:], b3_ap, None, OP.add)
            f_tiles.append(f)

            z_cur = state["z_cur"]
            if st < 3:
                c = 0.5 * H if st < 2 else H
                nc.vector.scalar_tensor_tensor(
                    zcx[0:64, :], f[:], float(c), z_cur[:], OP.mult, OP.add)
            else:
                t3 = hpool.tile([64, BC], F32, tag="t3", bufs=2, name="t3")
                nc.vector.tensor_add(t3[:], t2[:], f[:])
                nc.vector.scalar_tensor_tensor(
                    zcx[0:64, :], t3[:], float(H / 6.0), z_cur[:],
                    OP.mult, OP.add)
                nc.vector.scalar_tensor_tensor(
                    state["z_nxt"][:], t3[:], float(H / 6.0), z_cur[:],
                    OP.mult, OP.add)
                state["z_cur"], state["z_nxt"] = state["z_nxt"], state["z_cur"]

            # layer-2 tangent gates (feed the L3 tangent matmuls)
            d_prev = []
            for j in range(NJ):
                dj = hpool.tile([128, BC], F32R, tag="d", bufs=8, name="d")
                nc.vector.scalar_tensor_tensor(
                    dj[:], l2_q[j][:], -1.0, l2_pt[j][:], OP.add, OP.mult)
                d_prev.append(dj)

            # hoisted next-stage tangent matmuls cover the dh2/z waits
            if s + 1 < NSTG:
                psds_cur = emit_psds(s + 1)

            # ---- layer 3 tangent + divergence product ----
            psj = psum.tile([64, BC], F32, tag="ps", bufs=8, name="ps")
            for k in range(NJ):
                nc.tensor.matmul(psj[:], w3k(k), d_prev[k][:],
                                 start=(k == 0), stop=(k == NJ - 1))
            prod = hpool.tile([64, BC], F32R, tag="prod", bufs=3, name="prod")
            nc.vector.tensor_tensor(prod[:], psj[:], e_f, OP.mult)
            pending_div = (prod, st)

            # start the RK4 combine early to shorten the interval tail
            if st == 2:
                t1 = hpool.tile([64, BC], F32, tag="t1", bufs=2, name="t1")
                nc.vector.tensor_add(t1[:], f_tiles[1][:], f_tiles[2][:])
                t2 = hpool.tile([64, BC], F32, tag="t2", bufs=2, name="t2")
                nc.vector.scalar_tensor_tensor(
                    t2[:], t1[:], 2.0, f_tiles[0][:], OP.mult, OP.add)
            if st == 3:
                f_tiles = []

        emit_div(pending_div)

        nc.gpsimd.dma_start(zout_d[:], state["z_cur"][:])
        nc.gpsimd.dma_start(lpout_d[:], state["lp_cur"][:])

    nc.compile()
    return nc


def _prep_host(inputs):
    """Host-side packing: weights/biases shared across cores, per-core slices."""
    f32 = np.float32
    x = np.asarray(inputs["x"], f32)
    context = np.asarray(inputs["context"], f32)
    W0 = np.asarray(inputs["W0"], f32)
    b0 = np.asarray(inputs["b0"], f32)
    W1 = np.asarray(inputs["W1"], f32)
    b1 = np.asarray(inputs["b1"], f32)
    W2 = np.asarray(inputs["W2"], f32)
    b2 = np.asarray(inputs["b2"], f32)
    W3 = np.asarray(inputs["W3"], f32)
    b3 = np.asarray(inputs["b3"], f32)
    out_scale = np.asarray(inputs["out_scale"], f32)
    eps = np.asarray(inputs["eps"], f32)

    W3s = (W3 * out_scale).astype(f32)
    b3s = (b3 * out_scale).astype(f32)

    wpack = np.zeros((128, WPACK_COLS), f32)
    wpack[:, _C_W0:_C_W0 + 512] = W0[:128]
    wpack[:, _C_W1:_C_W1 + 2048] = W1.reshape(4, 128, 512).transpose(1, 0, 2).reshape(128, 2048)
    wpack[:, _C_W2:_C_W2 + 2048] = W2.reshape(4, 128, 512).transpose(1, 0, 2).reshape(128, 2048)
    wpack[:, _C_W3:_C_W3 + 256] = W3s.reshape(4, 128, 64).transpose(1, 0, 2).reshape(128, 256)
    w0t = W0[DIM + COND].astype(np.float64)
    for s, t in enumerate(_stage_times()):
        wpack[:, _C_TB + 4 * s:_C_TB + 4 * s + 4] = \
            (b0.astype(np.float64) + t * w0t).astype(f32).reshape(4, 128).T
    wpack[:, _C_BB:_C_BB + 4] = b1.reshape(4, 128).T
    wpack[:, _C_BB + 4:_C_BB + 8] = b2.reshape(4, 128).T
    wpack[0:64, _C_B3] = b3s
    # divergence coeffs: lp += cvec^T (jv' * e) with jv' = -jv:
    # lp_contrib = (h/6)*c*k_l = -(h/6)*c*div = +(h/6)*c*div'  => (H/6)*c.
    wpack[0:64, _C_CV] = f32(H / 6.0)
    wpack[0:64, _C_CV + 1] = f32((H / 6.0) * 2.0)

    in_maps = []
    for c in range(NCORES):
        sl = slice(c * BC, (c + 1) * BC)
        wp = wpack.copy()
        wp[0:64, _C_ZC:_C_ZC + 512] = x[sl].T
        wp[64:128, _C_ZC:_C_ZC + 512] = context[sl].T
        epsT = np.ascontiguousarray(
            eps[:, :, 0, sl, :].reshape(NSTG, BC, 64).transpose(2, 0, 1).reshape(64, NSTG * BC))
        in_maps.append({"wpack": wp, "epsT": epsT})
    return in_maps


_NC_CACHE = None


def kernel(**inputs):
    global _NC_CACHE
    if _NC_CACHE is None:
        _NC_CACHE = build_nc()
    nc = _NC_CACHE
    in_maps = _prep_host(inputs)
    res = run_bass_kernel_spmd(nc, in_maps, core_ids=list(range(NCORES)))
    z1 = np.empty((B, DIM), np.float32)
    lp1 = np.empty((B, 1), np.float32)
    for c in range(NCORES):
        sl = slice(c * BC, (c + 1) * BC)
        z1[sl] = res.results[c]["z_out"].T
        lp1[sl] = res.results[c]["lp_out"].T
    return z1, lp1
